# revision 11
# baseline (speedup 1.0000x reference)
"""Trainium2 Bass kernel for nn_MultiHeadAttention_68659347194437.

Spatial attention over the W axis (no softmax) with 1x1-conv projections:
    qp = wq*q + bq ; kp, vp likewise            (C=32 channels)
    attn = qp @ kp^T  per (b,h)                 [512, 512]
    att  = attn @ vp                            [512, 32]
    out  = att^T + q                            (NCHW residual)

No softmax -> the [512,512] score matrix collapses associatively:
    out_h = (A @ M1_h + [I;0])^T @ Qaug_h,   per head h, where
    A  = [wq|bq]^T [wk|bk]                 [33,33]  (host-precomputed)
    M1 = Kaug^T Vaug Pv^T = Gt^T Pv^T      [33,32]  (Gt = Vaug Kaug^T)
The +I folds the residual; row 32 of A@M1 is a per-head output bias.

Device schedule: 16 blocks of 4 heads. Per block the final matmul is ONE
K=128 matmul with a block-diagonal [128,128] weight (4 heads stacked),
streaming a [128,512] stacked-q tile. A depth-4 software pipeline keeps
the tensor engine from waiting on the PSUM->SBUF copies.

Sharding: data-parallel over batch B=8 across 8 NeuronCores, no comms.
Host-side work is pure relayout/packing of inputs and tiny weight algebra.
"""

import os
import numpy as np

import concourse.bass as bass
import concourse.bacc as bacc
import concourse.tile as tile
import concourse.mybir as mybir
from concourse.bass_utils import run_bass_kernel_spmd

B, C, H, W = 8, 32, 64, 512
CA = C + 1           # 33, augmented channel dim
NBLK = H // 4        # 16 blocks of 4 heads
NCH = 4 * 4          # chunks per block (4 heads x 4 chunks of 128 pixels)
BLK_GROUPS = [1, 1, 2, 4, 4, 4]      # DMA group sizes in blocks
PIPE = 3             # out-stage lags G-stage by 3 iterations

# "k65": W-block built with one K=65 matmul per head ([A_w^T; I] stacked)
# "iadd": fallback, identity added with a separate K=32 matmul
WMODE = os.environ.get("KERNEL_WMODE", "k65")

last_exec_time_ns = None

_cache = {}

_BF16_NP = np.dtype(mybir.dt.np(mybir.dt.bfloat16))


def _build(wmode):
    bf16 = mybir.dt.bfloat16
    f32 = mybir.dt.float32

    nc = bacc.Bacc(
        "TRN2",
        target_bir_lowering=False,
        debug=False,
        enable_asserts=False,
        num_devices=8,
    )

    qs_d = nc.dram_tensor("qs", [128, NBLK * W], bf16, kind="ExternalInput")
    kt_d = nc.dram_tensor("kt", [128, NBLK * NCH * CA], bf16, kind="ExternalInput")
    vt_d = nc.dram_tensor("vt", [128, NBLK * NCH * CA], bf16, kind="ExternalInput")
    # packed consts: cols 0:32 awi ([A_w^T; 0; I32], 96 rows), 32 alast
    # (A[32,:]), 33:65 pvt ([wv^T; bv]), 65:97 I32
    cst_d = nc.dram_tensor("cst", [96, 97], bf16, kind="ExternalInput")
    out_d = nc.dram_tensor("out", [128, NBLK * W], bf16, kind="ExternalOutput")

    qs_a = qs_d.ap()
    kt_a = kt_d.ap()
    vt_a = vt_d.ap()
    out_a = out_d.ap()

    # first block index of each DMA group, per block -> group
    gb0 = []
    acc = 0
    for n in BLK_GROUPS:
        gb0.append(acc)
        acc += n
    blk2g = []
    for g, n in enumerate(BLK_GROUPS):
        blk2g += [g] * n

    with tile.TileContext(nc) as tc:
        with (
            tc.tile_pool(name="const", bufs=1) as cpool,
            tc.tile_pool(name="inp", bufs=1) as inpool,
            tc.tile_pool(name="small", bufs=3) as spool,
            tc.tile_pool(name="outp", bufs=8) as opool,
            tc.tile_pool(name="psg", bufs=2, space=bass.MemorySpace.PSUM) as psg,
            tc.tile_pool(name="psm", bufs=2, space=bass.MemorySpace.PSUM) as psm,
            tc.tile_pool(name="psw", bufs=2, space=bass.MemorySpace.PSUM) as psw,
            tc.tile_pool(name="pso", bufs=2, space=bass.MemorySpace.PSUM) as pso,
        ):
            # ---- constants (one small DMA on the gpsimd queue) ----
            cst = cpool.tile([96, 97], bf16)
            nc.gpsimd.dma_start(cst[:], cst_d.ap()[:])
            awi = cst[:, 0:32]          # [96, 32]: [A_w^T; 0; I32]
            awt = cst[0:33, 0:32]       # [33, 32] (A_w^T)
            alast = cst[0:33, 32:33]    # [33, 1]
            pvt = cst[0:33, 33:65]      # [33, 32]
            i32 = cst[0:32, 65:97]      # [32, 32]

            # persistent double-buffered [M1; 0; I32] and W-block tiles
            m1a = []
            wb = []
            for j in range(2):
                t = cpool.tile([96, 128], bf16, name=f"m1a{j}")
                m1a.append(t)
                t2 = cpool.tile([128, 128], bf16, name=f"wb{j}")
                wb.append(t2)
                nc.vector.memset(t2[:], 0.0)
                nc.vector.memset(t[:], 0.0)
            if wmode == "k65":
                for j in range(2):
                    for i in range(4):
                        nc.vector.tensor_copy(
                            m1a[j][64:96, 32 * i:32 * (i + 1)], cst[64:96, 0:32]
                        )

            # ---- input DMA kicks, all up front, one stream per queue ----
            ktg, vtg, qsg = [], [], []
            for g, n in enumerate(BLK_GROUPS):
                c0 = gb0[g] * NCH * CA
                cn = n * NCH * CA
                t = inpool.tile([128, cn], bf16, tag=f"ktg{g}", name=f"ktg{g}")
                nc.sync.dma_start(t[:], kt_a[:, c0:c0 + cn])
                ktg.append(t)
            for g, n in enumerate(BLK_GROUPS):
                c0 = gb0[g] * NCH * CA
                cn = n * NCH * CA
                t = inpool.tile([128, cn], bf16, tag=f"vtg{g}", name=f"vtg{g}")
                nc.scalar.dma_start(t[:], vt_a[:, c0:c0 + cn])
                vtg.append(t)
            for g, n in enumerate(BLK_GROUPS):
                c0 = gb0[g] * W
                cn = n * W
                t = inpool.tile([128, cn], bf16, tag=f"qsg{g}", name=f"qsg{g}")
                qsg.append(t)
                if g < 3:
                    nc.gpsimd.dma_start(t[:], qs_a[:, c0:c0 + cn])

            # ---- pipelined main loop ----
            gts = [None] * NBLK      # gt SBUF tiles
            wps = [None] * NBLK      # W+bias PSUM tiles
            biass = [None] * NBLK    # bias SBUF tiles
            osbs = [None] * NBLK     # out SBUF (paired) tiles

            for it in range(NBLK + PIPE):
                b_m = it - 1   # M1 stage
                b_w = it - 2   # W + bias stage
                b_o = it - 3   # out stage
                b_g = it       # G stage

                # deferred qs kicks so gpsimd's early diag copies aren't
                # stuck behind a wall of DMA_DIRECT2D instructions
                if 1 <= it <= 3:
                    g = it + 2
                    c0 = gb0[g] * W
                    nc.gpsimd.dma_start(
                        qsg[g][:], qs_a[:, c0:c0 + BLK_GROUPS[g] * W]
                    )

                # M1: m1_ps = Gt_i^T @ pvt for 4 heads
                if 0 <= b_m < NBLK:
                    m1_ps = psm.tile([33, 128], f32, tag="m1")
                    gt = gts[b_m]
                    for i in range(4):
                        nc.tensor.matmul(
                            m1_ps[:, 32 * i:32 * (i + 1)],
                            gt[:, CA * i:CA * (i + 1)],
                            pvt,
                        )
                    nc.vector.tensor_copy(m1a[b_m % 2][0:33, 0:128], m1_ps[:])

                # W-block (block-diag L+I) and bias
                if 0 <= b_w < NBLK:
                    w_ps = psw.tile([128, 132], f32, tag="w")
                    wps[b_w] = w_ps
                    ma = m1a[b_w % 2]
                    for i in range(4):
                        sl = w_ps[32 * i:32 * (i + 1), 32 * i:32 * (i + 1)]
                        if wmode == "k65":
                            nc.tensor.matmul(
                                sl, awi, ma[:, 32 * i:32 * (i + 1)],
                                tile_position=(0, 32 * i),
                            )
                        else:
                            nc.tensor.matmul(
                                sl, i32, i32,
                                start=True, stop=False,
                                tile_position=(0, 32 * i),
                            )
                            nc.tensor.matmul(
                                sl, awt, ma[0:33, 32 * i:32 * (i + 1)],
                                start=False, stop=True,
                                tile_position=(0, 32 * i),
                            )
                    # bias[32i+co] = sum_ck A[32,ck] * M1_i[ck,co]
                    nc.tensor.matmul(w_ps[:, 128:129], ma[0:33, :], alast)
                    wtile = wb[b_w % 2]
                    for i in range(4):
                        nc.scalar.copy(
                            wtile[32 * i:32 * (i + 1), 32 * i:32 * (i + 1)],
                            w_ps[32 * i:32 * (i + 1), 32 * i:32 * (i + 1)],
                        )
                    bias_sb = spool.tile([128, 1], f32, tag="bias")
                    biass[b_w] = bias_sb
                    nc.vector.tensor_copy(bias_sb[:], w_ps[:, 128:129])

                # out: one K=128 matmul for 4 heads, then +bias on vector
                if 0 <= b_o < NBLK:
                    g = blk2g[b_o]
                    o_ps = pso.tile([128, W], f32, tag="o")
                    qc0 = (b_o - gb0[g]) * W
                    nc.tensor.matmul(
                        o_ps[:], wb[b_o % 2][:], qsg[g][:, qc0:qc0 + W]
                    )
                    if b_o % 2 == 0:
                        osb = opool.tile([128, 2 * W], bf16, tag="osb")
                        osbs[b_o] = osb
                        half = osb[:, 0:W]
                    else:
                        osb = osbs[b_o - 1]
                        half = osb[:, W:2 * W]
                    nc.vector.tensor_scalar_add(half, o_ps[:], biass[b_o][:])
                    if b_o % 2 == 1:
                        eng = nc.sync if b_o < 8 else nc.scalar
                        eng.dma_start(
                            out_a[:, (b_o - 1) * W:(b_o + 1) * W], osb[:]
                        )

                # G: Gt accumulation over 4 chunks per head
                if b_g < NBLK:
                    g = blk2g[b_g]
                    g_ps = psg.tile([33, 4 * CA], f32, tag="g")
                    gt_sb = spool.tile([33, 4 * CA], bf16, tag="gt")
                    gts[b_g] = gt_sb
                    for i in range(4):
                        for j in range(4):
                            ch = (b_g - gb0[g]) * NCH + i * 4 + j
                            o = ch * CA
                            nc.tensor.matmul(
                                g_ps[:, CA * i:CA * (i + 1)],
                                vtg[g][:, o:o + CA],
                                ktg[g][:, o:o + CA],
                                start=(j == 0),
                                stop=(j == 3),
                            )
                    nc.vector.tensor_copy(gt_sb[:], g_ps[:])

    nc.compile()
    return nc


def _prep_core(qb, kb, vb):
    """Host relayout for one batch element: qs [128,8192], kt/vt [128,8448]."""
    qs = np.ascontiguousarray(
        qb.reshape(C, NBLK, 4, W).transpose(2, 0, 1, 3)
    ).reshape(128, NBLK * W).astype(_BF16_NP)

    def tr(x):
        t = np.empty((H * W, CA), dtype=np.float32)
        t[:, :C] = x.reshape(C, H * W).T
        t[:, C] = 1.0
        return np.ascontiguousarray(
            t.reshape(NBLK * NCH, 128, CA).transpose(1, 0, 2)
        ).reshape(128, NBLK * NCH * CA).astype(_BF16_NP)

    return qs, tr(kb), tr(vb)


def _install_ntff_hook():
    """Provide antenv.axon_hooks (absent in this image) so trace=True works."""
    import sys
    import types

    if "antenv.axon_hooks" in sys.modules:
        return
    try:
        import antenv
    except ImportError:
        return
    mod = types.ModuleType("antenv.axon_hooks")
    store = {}
    mod.set_axon_ntff_profile_hook = lambda h: store.__setitem__("h", h)
    mod.get_axon_ntff_profile_hook = lambda: store.get("h")
    sys.modules["antenv.axon_hooks"] = mod
    antenv.axon_hooks = mod
    try:
        from trn_agent_boot.trn_boot import _ntff_profile_via_ctypes

        hook = _ntff_profile_via_ctypes("/opt/axon/libaxon_pjrt.so")
        if hook is not None:
            store["h"] = hook
    except Exception:
        pass


def kernel(q, k, v, wq, bq, wk, bk, wv, bv):
    global last_exec_time_ns
    if WMODE not in _cache:
        _cache[WMODE] = _build(WMODE)
    nc = _cache[WMODE]

    q = np.asarray(q, np.float32)
    k = np.asarray(k, np.float32)
    v = np.asarray(v, np.float32)
    wq = np.asarray(wq, np.float32)
    bq = np.asarray(bq, np.float32)
    wk = np.asarray(wk, np.float32)
    bk = np.asarray(bk, np.float32)
    wv = np.asarray(wv, np.float32)
    bv = np.asarray(bv, np.float32)

    # A = [wq|bq]^T @ [wk|bk]  (33x33), host-side weight algebra
    wqb = np.concatenate([wq, bq[:, None]], axis=1)  # [32, 33]
    wkb = np.concatenate([wk, bk[:, None]], axis=1)
    A = wqb.T @ wkb                                   # [33, 33]
    cst = np.zeros((96, 97), dtype=np.float32)
    cst[0:33, 0:32] = A[0:32, :].T                    # awt
    cst[64:96, 0:32] = np.eye(32)                     # I32 (awi lower band)
    cst[0:33, 32] = A[32, :]                          # alast
    cst[0:33, 33:65] = np.concatenate([wv.T, bv[None, :]], axis=0)  # pvt
    cst[0:32, 65:97] = np.eye(32)                     # i32 const
    cst = cst.astype(_BF16_NP)

    in_maps = []
    for b in range(B):
        qs, kt, vt = _prep_core(q[b], k[b], v[b])
        in_maps.append({"qs": qs, "kt": kt, "vt": vt, "cst": cst})

    trace = os.environ.get("KERNEL_TRACE", "0") == "1"
    if trace:
        _install_ntff_hook()
    res = run_bass_kernel_spmd(nc, in_maps, core_ids=list(range(B)), trace=trace)
    last_exec_time_ns = res.exec_time_ns

    outs = []
    for b in range(B):
        arr = np.asarray(res.results[b]["out"], dtype=np.float32)
        arr = arr.reshape(4, C, NBLK, W).transpose(1, 2, 0, 3).reshape(C, H, W)
        outs.append(arr)
    return np.stack(outs).astype(np.float32)


# revision 13
# speedup vs baseline: 1.3593x; 1.3593x over previous
"""Trainium2 Bass kernel for nn_MultiHeadAttention_68659347194437.

Spatial attention over the W axis (no softmax) with 1x1-conv projections:
    qp = wq*q + bq ; kp, vp likewise            (C=32 channels)
    attn = qp @ kp^T  per (b,h)                 [512, 512]
    att  = attn @ vp                            [512, 32]
    out  = att^T + q                            (NCHW residual)

No softmax -> the [512,512] score matrix collapses associatively:
    out_h = (A @ M1_h + [I;0])^T @ Qaug_h,   per head h, where
    A  = [wq|bq]^T [wk|bk]                 [33,33]  (host-precomputed)
    M1 = Gt^T Pv^T                         [33,32]  (Gt = Vaug Kaug^T)
The +I folds the residual; row 32 of A@M1 is a per-head output bias.

Device schedule: 8 pipeline iterations of 2 blocks (8 heads). Per block
the final matmul is ONE K=128 matmul with a block-diagonal [128,128]
weight (4 heads stacked) streaming a [128,512] stacked-q tile. The
block-diag weight PSUM is initialized with an identity matmul (zeros
off-diagonal + the residual +I in one PE op), so each PSUM->SBUF drain
is a single wide copy per pair -- DVE ops have ~300ns fixed cost, so
copies are batched aggressively. The per-head bias column rides in the
same copy and feeds the scalar-engine ACTIVATE that drains the output.

Sharding: data-parallel over batch B=8 across 8 NeuronCores, no comms.
Host-side work is pure relayout/packing of inputs and tiny weight algebra.
"""

import os
import numpy as np

import concourse.bass as bass
import concourse.bacc as bacc
import concourse.tile as tile
import concourse.mybir as mybir
from concourse.bass_utils import run_bass_kernel_spmd

B, C, H, W = 8, 32, 64, 512
CA = C + 1           # 33, augmented channel dim
NBLK = H // 4        # 16 blocks of 4 heads
NPAIR = NBLK // 2    # 8 pipeline iterations of 2 blocks
NCH = 4 * 4          # chunks per block (4 heads x 4 chunks of 128 pixels)
PAIR_GROUPS = [1, 1, 2, 2, 2]        # DMA group sizes in block-pairs
PIPE = 3             # out-stage lags G-stage by 3 iterations

last_exec_time_ns = None

_cache = {}

_BF16_NP = np.dtype(mybir.dt.np(mybir.dt.bfloat16))


def _build():
    bf16 = mybir.dt.bfloat16
    f32 = mybir.dt.float32

    nc = bacc.Bacc(
        "TRN2",
        target_bir_lowering=False,
        debug=False,
        enable_asserts=False,
        num_devices=8,
    )

    qs_d = nc.dram_tensor("qs", [128, NBLK * W], bf16, kind="ExternalInput")
    kt_d = nc.dram_tensor("kt", [128, NBLK * NCH * CA], bf16, kind="ExternalInput")
    vt_d = nc.dram_tensor("vt", [128, NBLK * NCH * CA], bf16, kind="ExternalInput")
    # packed consts: cols 0:32 awt (A[0:32,:]^T), 32 alast (A[32,:]),
    # 33:65 pvt ([wv^T; bv]), 65:193 I128, 193:197 zeros
    cst_d = nc.dram_tensor("cst", [128, 197], bf16, kind="ExternalInput")
    out_d = nc.dram_tensor("out", [128, NBLK * W], bf16, kind="ExternalOutput")

    qs_a = qs_d.ap()
    kt_a = kt_d.ap()
    vt_a = vt_d.ap()
    out_a = out_d.ap()

    gp0 = []
    acc = 0
    for n in PAIR_GROUPS:
        gp0.append(acc)
        acc += n
    pair2g = []
    for g, n in enumerate(PAIR_GROUPS):
        pair2g += [g] * n

    with tile.TileContext(nc) as tc:
        with (
            tc.tile_pool(name="const", bufs=1) as cpool,
            tc.tile_pool(name="inp", bufs=1) as inpool,
            tc.tile_pool(name="small", bufs=3) as spool,
            tc.tile_pool(name="outp", bufs=4) as opool,
            tc.tile_pool(name="psg", bufs=2, space=bass.MemorySpace.PSUM) as psg,
            tc.tile_pool(name="psm", bufs=2, space=bass.MemorySpace.PSUM) as psm,
            tc.tile_pool(name="psw", bufs=2, space=bass.MemorySpace.PSUM) as psw,
            tc.tile_pool(name="pso", bufs=2, space=bass.MemorySpace.PSUM) as pso,
        ):
            cst = cpool.tile([128, 197], bf16)
            nc.sync.dma_start(cst[:], cst_d.ap()[:])
            awt = cst[0:33, 0:32]       # [33, 32] (A_w^T)
            alast = cst[0:33, 32:33]    # [33, 1]
            pvt = cst[0:33, 33:65]      # [33, 32]
            i128 = cst[:, 65:193]       # [128, 128]
            iw = cst[:, 65:197]         # [128, 132]: [I128 | 0000]

            # ---- input DMA kicks, all up front ----
            # sync queue: cst, then kt/qs interleaved in need order
            # scalar queue: vt (out DMAs join later, in-loop)
            ktg, vtg, qsg = [], [], []
            for g, n in enumerate(PAIR_GROUPS):
                c0 = gp0[g] * 2 * NCH * CA
                cn = n * 2 * NCH * CA
                t = inpool.tile([128, cn], bf16, tag=f"ktg{g}", name=f"ktg{g}")
                ktg.append(t)
                c0q = gp0[g] * 2 * W
                cnq = n * 2 * W
                tq = inpool.tile([128, cnq], bf16, tag=f"qsg{g}", name=f"qsg{g}")
                qsg.append(tq)
                tv = inpool.tile([128, cn], bf16, tag=f"vtg{g}", name=f"vtg{g}")
                vtg.append(tv)
                nc.scalar.dma_start(tv[:], vt_a[:, c0:c0 + cn])
            for g, n in enumerate(PAIR_GROUPS):
                c0 = gp0[g] * 2 * NCH * CA
                cn = n * 2 * NCH * CA
                nc.sync.dma_start(ktg[g][:], kt_a[:, c0:c0 + cn])
                if g >= 1:
                    gq = g - 1
                    c0q = gp0[gq] * 2 * W
                    cnq = PAIR_GROUPS[gq] * 2 * W
                    nc.sync.dma_start(qsg[gq][:], qs_a[:, c0q:c0q + cnq])
            gq = len(PAIR_GROUPS) - 1
            c0q = gp0[gq] * 2 * W
            nc.sync.dma_start(qsg[gq][:], qs_a[:, c0q:c0q + PAIR_GROUPS[gq] * 2 * W])

            # ---- pipelined main loop over block-pairs ----
            gts = [None] * NPAIR     # gt SBUF tiles   [33, 264]
            m12s = [None] * NPAIR    # M1 SBUF tiles   [33, 256]
            wbs = [None] * NPAIR     # W-block SBUF    [128, 264]

            for it in range(NPAIR + PIPE):
                p_m = it - 1   # M1 stage
                p_w = it - 2   # W + bias stage
                p_o = it - 3   # out stage
                p_g = it       # G stage

                # M1: per head, m1 = Gt_h^T @ pvt
                if 0 <= p_m < NPAIR:
                    m1_ps = psm.tile([33, 256], f32, tag="m1")
                    gt = gts[p_m]
                    for hh in range(8):
                        nc.tensor.matmul(
                            m1_ps[:, 32 * hh:32 * (hh + 1)],
                            gt[:, CA * hh:CA * (hh + 1)],
                            pvt,
                        )
                    m12 = spool.tile([33, 256], bf16, tag="m12")
                    m12s[p_m] = m12
                    nc.vector.tensor_copy(m12[:], m1_ps[:])

                # W-blocks: I-init (zeros + residual identity), then
                # per-head diag L = A_w @ M1, plus bias columns
                if 0 <= p_w < NPAIR:
                    w_ps = psw.tile([128, 264], f32, tag="w")
                    m12 = m12s[p_w]
                    for blk in range(2):
                        cb = 132 * blk
                        nc.tensor.matmul(
                            w_ps[:, cb:cb + 132], i128, iw,
                            start=True, stop=False, skip_group_check=True,
                        )
                        for i in range(4):
                            nc.tensor.matmul(
                                w_ps[32 * i:32 * (i + 1), cb + 32 * i:cb + 32 * (i + 1)],
                                awt,
                                m12[:, 128 * blk + 32 * i:128 * blk + 32 * (i + 1)],
                                start=False, stop=True, skip_group_check=True,
                                tile_position=(0, 32 * i),
                            )
                        nc.tensor.matmul(
                            w_ps[:, cb + 128:cb + 129],
                            m12[:, 128 * blk:128 * (blk + 1)],
                            alast,
                            start=False, stop=True, skip_group_check=True,
                        )
                    wb = spool.tile([128, 264], bf16, tag="wb")
                    wbs[p_w] = wb
                    nc.vector.tensor_copy(wb[:], w_ps[:])

                # out: one K=128 matmul per block + ACTIVATE drain with bias
                if 0 <= p_o < NPAIR:
                    g = pair2g[p_o]
                    wb = wbs[p_o]
                    osb = opool.tile([128, 2 * W], bf16, tag="osb")
                    for blk in range(2):
                        b = 2 * p_o + blk
                        o_ps = pso.tile([128, W], f32, tag="o")
                        qc0 = (b - gp0[g] * 2) * W
                        nc.tensor.matmul(
                            o_ps[:],
                            wb[:, 132 * blk:132 * blk + 128],
                            qsg[g][:, qc0:qc0 + W],
                        )
                        nc.scalar.activation(
                            osb[:, W * blk:W * (blk + 1)],
                            o_ps[:],
                            mybir.ActivationFunctionType.Identity,
                            bias=wb[:, 132 * blk + 128:132 * blk + 129],
                        )
                    nc.scalar.dma_start(
                        out_a[:, 2 * p_o * W:2 * (p_o + 1) * W], osb[:]
                    )

                # G: Gt accumulation over 4 chunks per head, 8 heads
                if p_g < NPAIR:
                    g = pair2g[p_g]
                    g_ps = psg.tile([33, 264], f32, tag="g")
                    gt_sb = spool.tile([33, 264], bf16, tag="gt")
                    gts[p_g] = gt_sb
                    ch0 = (p_g - gp0[g]) * 2 * NCH
                    for hh in range(8):
                        for j in range(4):
                            o = (ch0 + hh * 4 + j) * CA
                            nc.tensor.matmul(
                                g_ps[:, CA * hh:CA * (hh + 1)],
                                vtg[g][:, o:o + CA],
                                ktg[g][:, o:o + CA],
                                start=(j == 0),
                                stop=(j == 3),
                            )
                    nc.vector.tensor_copy(gt_sb[:], g_ps[:])

    nc.compile()
    return nc


def _prep_core(qb, kb, vb):
    """Host relayout for one batch element: qs [128,8192], kt/vt [128,8448]."""
    qs = np.ascontiguousarray(
        qb.reshape(C, NBLK, 4, W).transpose(2, 0, 1, 3)
    ).reshape(128, NBLK * W).astype(_BF16_NP)

    def tr(x):
        t = np.empty((H * W, CA), dtype=np.float32)
        t[:, :C] = x.reshape(C, H * W).T
        t[:, C] = 1.0
        return np.ascontiguousarray(
            t.reshape(NBLK * NCH, 128, CA).transpose(1, 0, 2)
        ).reshape(128, NBLK * NCH * CA).astype(_BF16_NP)

    return qs, tr(kb), tr(vb)


def _install_ntff_hook():
    """Provide antenv.axon_hooks (absent in this image) so trace=True works."""
    import sys
    import types

    if "antenv.axon_hooks" in sys.modules:
        return
    try:
        import antenv
    except ImportError:
        return
    mod = types.ModuleType("antenv.axon_hooks")
    store = {}
    mod.set_axon_ntff_profile_hook = lambda h: store.__setitem__("h", h)
    mod.get_axon_ntff_profile_hook = lambda: store.get("h")
    sys.modules["antenv.axon_hooks"] = mod
    antenv.axon_hooks = mod
    try:
        from trn_agent_boot.trn_boot import _ntff_profile_via_ctypes

        hook = _ntff_profile_via_ctypes("/opt/axon/libaxon_pjrt.so")
        if hook is not None:
            store["h"] = hook
    except Exception:
        pass


def kernel(q, k, v, wq, bq, wk, bk, wv, bv):
    global last_exec_time_ns
    if "nc" not in _cache:
        _cache["nc"] = _build()
    nc = _cache["nc"]

    q = np.asarray(q, np.float32)
    k = np.asarray(k, np.float32)
    v = np.asarray(v, np.float32)
    wq = np.asarray(wq, np.float32)
    bq = np.asarray(bq, np.float32)
    wk = np.asarray(wk, np.float32)
    bk = np.asarray(bk, np.float32)
    wv = np.asarray(wv, np.float32)
    bv = np.asarray(bv, np.float32)

    # A = [wq|bq]^T @ [wk|bk]  (33x33), host-side weight algebra
    wqb = np.concatenate([wq, bq[:, None]], axis=1)  # [32, 33]
    wkb = np.concatenate([wk, bk[:, None]], axis=1)
    A = wqb.T @ wkb                                   # [33, 33]
    cst = np.zeros((128, 197), dtype=np.float32)
    cst[0:33, 0:32] = A[0:32, :].T                    # awt
    cst[0:33, 32] = A[32, :]                          # alast
    cst[0:33, 33:65] = np.concatenate([wv.T, bv[None, :]], axis=0)  # pvt
    cst[0:128, 65:193] = np.eye(128)                  # I128
    cst = cst.astype(_BF16_NP)

    in_maps = []
    for b in range(B):
        qs, kt, vt = _prep_core(q[b], k[b], v[b])
        in_maps.append({"qs": qs, "kt": kt, "vt": vt, "cst": cst})

    trace = os.environ.get("KERNEL_TRACE", "0") == "1"
    if trace:
        _install_ntff_hook()
    res = run_bass_kernel_spmd(nc, in_maps, core_ids=list(range(B)), trace=trace)
    last_exec_time_ns = res.exec_time_ns

    outs = []
    for b in range(B):
        arr = np.asarray(res.results[b]["out"], dtype=np.float32)
        arr = arr.reshape(4, C, NBLK, W).transpose(1, 2, 0, 3).reshape(C, H, W)
        outs.append(arr)
    return np.stack(outs).astype(np.float32)


# revision 18
# speedup vs baseline: 1.3854x; 1.0192x over previous
"""Trainium2 Bass kernel for nn_MultiHeadAttention_68659347194437.

Spatial attention over the W axis (no softmax) with 1x1-conv projections:
    qp = wq*q + bq ; kp, vp likewise            (C=32 channels)
    attn = qp @ kp^T  per (b,h)                 [512, 512]
    att  = attn @ vp                            [512, 32]
    out  = att^T + q                            (NCHW residual)

No softmax -> the [512,512] score matrix collapses associatively:
    out_h = (A @ M1_h + [I;0])^T @ Qaug_h,   per head h, where
    A  = [wq|bq]^T [wk|bk]                 [33,33]  (host-precomputed)
    M1 = Gt^T Pv^T                         [33,32]  (Gt = Vaug Kaug^T)
The +I folds the residual; row 32 of A@M1 is a per-head output bias.

Device schedule: 8 pipeline iterations of 2 blocks (8 heads). Per block
the final matmul is ONE K=128 matmul with a block-diagonal [128,128]
weight (4 heads stacked) streaming a [128,512] stacked-q tile. The
block-diag weight PSUM is initialized with an identity matmul (zeros
off-diagonal + the residual +I in one PE op), so each PSUM->SBUF drain
is a single wide copy per pair -- DVE ops have ~300ns fixed cost, so
copies are batched aggressively. The per-head bias column rides in the
same copy and feeds the scalar-engine ACTIVATE that drains the output.

Sharding: data-parallel over batch B=8 across 8 NeuronCores, no comms.
Host-side work is pure relayout/packing of inputs and tiny weight algebra.
"""

import os
import numpy as np

import concourse.bass as bass
import concourse.bacc as bacc
import concourse.tile as tile
import concourse.mybir as mybir
from concourse.bass_utils import run_bass_kernel_spmd

B, C, H, W = 8, 32, 64, 512
CA = C + 1           # 33, augmented channel dim
NBLK = H // 4        # 16 blocks of 4 heads
NPAIR = NBLK // 2    # 8 pipeline iterations of 2 blocks
NCH = 4 * 4          # chunks per block (4 heads x 4 chunks of 128 pixels)
PAIR_GROUPS = [1, 1, 2, 2, 2]        # DMA group sizes in block-pairs
PIPE = 3             # out-stage lags G-stage by 3 iterations

last_exec_time_ns = None

_cache = {}

_BF16_NP = np.dtype(mybir.dt.np(mybir.dt.bfloat16))


def _build():
    bf16 = mybir.dt.bfloat16
    f32 = mybir.dt.float32

    nc = bacc.Bacc(
        "TRN2",
        target_bir_lowering=False,
        debug=False,
        enable_asserts=False,
        num_devices=8,
    )

    qs_d = nc.dram_tensor("qs", [128, NBLK * W], bf16, kind="ExternalInput")
    kt_d = nc.dram_tensor("kt", [128, NBLK * NCH * CA], bf16, kind="ExternalInput")
    vt_d = nc.dram_tensor("vt", [128, NBLK * NCH * CA], bf16, kind="ExternalInput")
    # packed consts: cols 0:32 awt (A[0:32,:]^T), 32 alast (A[32,:]),
    # 33:65 pvt ([wv^T; bv]), 65:329 [I128|0|I128|0] (132-col stride)
    cst_d = nc.dram_tensor("cst", [128, 329], bf16, kind="ExternalInput")
    out_d = nc.dram_tensor("out", [128, NBLK * W], bf16, kind="ExternalOutput")

    qs_a = qs_d.ap()
    kt_a = kt_d.ap()
    vt_a = vt_d.ap()
    out_a = out_d.ap()

    gp0 = []
    acc = 0
    for n in PAIR_GROUPS:
        gp0.append(acc)
        acc += n
    pair2g = []
    for g, n in enumerate(PAIR_GROUPS):
        pair2g += [g] * n

    with tile.TileContext(nc) as tc:
        with (
            tc.tile_pool(name="const", bufs=1) as cpool,
            tc.tile_pool(name="inp", bufs=1) as inpool,
            tc.tile_pool(name="small", bufs=3) as spool,
            tc.tile_pool(name="outp", bufs=4) as opool,
            tc.tile_pool(name="psg", bufs=2, space=bass.MemorySpace.PSUM) as psg,
            tc.tile_pool(name="psm", bufs=1, space=bass.MemorySpace.PSUM) as psm,
            tc.tile_pool(name="psw", bufs=2, space=bass.MemorySpace.PSUM) as psw,
            tc.tile_pool(name="pso", bufs=3, space=bass.MemorySpace.PSUM) as pso,
        ):
            cst = cpool.tile([128, 329], bf16)
            awt = cst[0:33, 0:32]       # [33, 32] (A_w^T)
            alast = cst[0:33, 32:33]    # [33, 1]
            pvt = cst[0:33, 33:65]      # [33, 32]
            i128 = cst[:, 65:193]       # [128, 128]
            ii2 = cst[:, 65:329]        # [128, 264]: [I128|0|I128|0]

            # ---- input DMA kicks, all up front, in need order ----
            # sync queue: kt0, cst, kt1.., qs interleaved
            # scalar queue: vt (out DMAs join later, in-loop)
            ktg, vtg, qsg = [], [], []
            for g, n in enumerate(PAIR_GROUPS):
                c0 = gp0[g] * 2 * NCH * CA
                cn = n * 2 * NCH * CA
                t = inpool.tile([128, cn], bf16, tag=f"ktg{g}", name=f"ktg{g}")
                ktg.append(t)
                cnq = n * 2 * W
                tq = inpool.tile([128, cnq], bf16, tag=f"qsg{g}", name=f"qsg{g}")
                qsg.append(tq)
                tv = inpool.tile([128, cn], bf16, tag=f"vtg{g}", name=f"vtg{g}")
                vtg.append(tv)
                nc.scalar.dma_start(tv[:], vt_a[:, c0:c0 + cn])

            def kick_kt(g):
                c0 = gp0[g] * 2 * NCH * CA
                nc.sync.dma_start(
                    ktg[g][:], kt_a[:, c0:c0 + PAIR_GROUPS[g] * 2 * NCH * CA]
                )

            def kick_qs(g):
                c0 = gp0[g] * 2 * W
                nc.sync.dma_start(
                    qsg[g][:], qs_a[:, c0:c0 + PAIR_GROUPS[g] * 2 * W]
                )

            kick_kt(0)
            nc.sync.dma_start(cst[:], cst_d.ap()[:])
            kick_kt(1)
            kick_kt(2)
            kick_qs(0)
            kick_kt(3)
            kick_qs(1)
            kick_kt(4)
            kick_qs(2)
            kick_qs(3)
            kick_qs(4)

            # ---- pipelined main loop over block-pairs ----
            gts = [None] * NPAIR     # gt SBUF tiles   [33, 264]
            m12s = [None] * NPAIR    # M1 SBUF tiles   [33, 256]
            wbs = [None] * NPAIR     # W-block SBUF    [128, 264]

            for it in range(NPAIR + PIPE):
                p_m = it - 1   # M1 stage
                p_w = it - 2   # W + bias stage
                p_o = it - 3   # out stage
                p_g = it       # G stage

                # M1: per head, m1 = Gt_h^T @ pvt
                if 0 <= p_m < NPAIR:
                    m1_ps = psm.tile([33, 256], f32, tag="m1")
                    gt = gts[p_m]
                    for hh in range(8):
                        nc.tensor.matmul(
                            m1_ps[:, 32 * hh:32 * (hh + 1)],
                            gt[:, CA * hh:CA * (hh + 1)],
                            pvt,
                        )
                    m12 = spool.tile([33, 256], bf16, tag="m12")
                    m12s[p_m] = m12
                    nc.vector.tensor_copy(m12[:], m1_ps[:])

                # W-blocks: I-init (zeros + residual identity), then
                # per-head diag L = A_w @ M1, plus bias columns
                if 0 <= p_w < NPAIR:
                    w_ps = psw.tile([128, 264], f32, tag="w")
                    m12 = m12s[p_w]
                    nc.tensor.matmul(
                        w_ps[:], i128, ii2,
                        start=True, stop=False, skip_group_check=True,
                    )
                    for blk in range(2):
                        cb = 132 * blk
                        for i in range(4):
                            nc.tensor.matmul(
                                w_ps[32 * i:32 * (i + 1), cb + 32 * i:cb + 32 * (i + 1)],
                                awt,
                                m12[:, 128 * blk + 32 * i:128 * blk + 32 * (i + 1)],
                                start=False, stop=True, skip_group_check=True,
                                tile_position=(0, 32 * i),
                            )
                        nc.tensor.matmul(
                            w_ps[:, cb + 128:cb + 129],
                            m12[:, 128 * blk:128 * (blk + 1)],
                            alast,
                            start=False, stop=True, skip_group_check=True,
                        )
                    wb = spool.tile([128, 264], bf16, tag="wb")
                    wbs[p_w] = wb
                    nc.vector.tensor_copy(wb[:], w_ps[:])

                # out: one K=128 matmul per block + ACTIVATE drain with bias
                if 0 <= p_o < NPAIR:
                    g = pair2g[p_o]
                    wb = wbs[p_o]
                    osb = opool.tile([128, 2 * W], bf16, tag="osb")
                    for blk in range(2):
                        b = 2 * p_o + blk
                        o_ps = pso.tile([128, W], f32, tag="o")
                        qc0 = (b - gp0[g] * 2) * W
                        nc.tensor.matmul(
                            o_ps[:],
                            wb[:, 132 * blk:132 * blk + 128],
                            qsg[g][:, qc0:qc0 + W],
                        )
                        nc.scalar.activation(
                            osb[:, W * blk:W * (blk + 1)],
                            o_ps[:],
                            mybir.ActivationFunctionType.Identity,
                            bias=wb[:, 132 * blk + 128:132 * blk + 129],
                        )
                    if p_o == NPAIR - 1:
                        # split the final flush across both hardware queues
                        nc.sync.dma_start(
                            out_a[:, 2 * p_o * W:(2 * p_o + 1) * W], osb[:, 0:W]
                        )
                        nc.scalar.dma_start(
                            out_a[:, (2 * p_o + 1) * W:2 * (p_o + 1) * W],
                            osb[:, W:2 * W],
                        )
                    else:
                        nc.scalar.dma_start(
                            out_a[:, 2 * p_o * W:2 * (p_o + 1) * W], osb[:]
                        )

                # G: Gt accumulation over 4 chunks per head, 8 heads
                if p_g < NPAIR:
                    g = pair2g[p_g]
                    g_ps = psg.tile([33, 264], f32, tag="g")
                    gt_sb = spool.tile([33, 264], bf16, tag="gt")
                    gts[p_g] = gt_sb
                    ch0 = (p_g - gp0[g]) * 2 * NCH
                    for hh in range(8):
                        for j in range(4):
                            o = (ch0 + hh * 4 + j) * CA
                            nc.tensor.matmul(
                                g_ps[:, CA * hh:CA * (hh + 1)],
                                vtg[g][:, o:o + CA],
                                ktg[g][:, o:o + CA],
                                start=(j == 0),
                                stop=(j == 3),
                            )
                    nc.vector.tensor_copy(gt_sb[:], g_ps[:])

    nc.compile()
    return nc


def _prep_core(qb, kb, vb):
    """Host relayout for one batch element: qs [128,8192], kt/vt [128,8448]."""
    qs = np.ascontiguousarray(
        qb.reshape(C, NBLK, 4, W).transpose(2, 0, 1, 3)
    ).reshape(128, NBLK * W).astype(_BF16_NP)

    def tr(x):
        t = np.empty((H * W, CA), dtype=np.float32)
        t[:, :C] = x.reshape(C, H * W).T
        t[:, C] = 1.0
        return np.ascontiguousarray(
            t.reshape(NBLK * NCH, 128, CA).transpose(1, 0, 2)
        ).reshape(128, NBLK * NCH * CA).astype(_BF16_NP)

    return qs, tr(kb), tr(vb)


def _install_ntff_hook():
    """Provide antenv.axon_hooks (absent in this image) so trace=True works."""
    import sys
    import types

    if "antenv.axon_hooks" in sys.modules:
        return
    try:
        import antenv
    except ImportError:
        return
    mod = types.ModuleType("antenv.axon_hooks")
    store = {}
    mod.set_axon_ntff_profile_hook = lambda h: store.__setitem__("h", h)
    mod.get_axon_ntff_profile_hook = lambda: store.get("h")
    sys.modules["antenv.axon_hooks"] = mod
    antenv.axon_hooks = mod
    try:
        from trn_agent_boot.trn_boot import _ntff_profile_via_ctypes

        hook = _ntff_profile_via_ctypes("/opt/axon/libaxon_pjrt.so")
        if hook is not None:
            store["h"] = hook
    except Exception:
        pass


def kernel(q, k, v, wq, bq, wk, bk, wv, bv):
    global last_exec_time_ns
    if "nc" not in _cache:
        _cache["nc"] = _build()
    nc = _cache["nc"]

    q = np.asarray(q, np.float32)
    k = np.asarray(k, np.float32)
    v = np.asarray(v, np.float32)
    wq = np.asarray(wq, np.float32)
    bq = np.asarray(bq, np.float32)
    wk = np.asarray(wk, np.float32)
    bk = np.asarray(bk, np.float32)
    wv = np.asarray(wv, np.float32)
    bv = np.asarray(bv, np.float32)

    # A = [wq|bq]^T @ [wk|bk]  (33x33), host-side weight algebra
    wqb = np.concatenate([wq, bq[:, None]], axis=1)  # [32, 33]
    wkb = np.concatenate([wk, bk[:, None]], axis=1)
    A = wqb.T @ wkb                                   # [33, 33]
    cst = np.zeros((128, 329), dtype=np.float32)
    cst[0:33, 0:32] = A[0:32, :].T                    # awt
    cst[0:33, 32] = A[32, :]                          # alast
    cst[0:33, 33:65] = np.concatenate([wv.T, bv[None, :]], axis=0)  # pvt
    cst[0:128, 65:193] = np.eye(128)                  # I128
    cst[0:128, 197:325] = np.eye(128)                 # second I128 for ii2
    cst = cst.astype(_BF16_NP)

    in_maps = []
    for b in range(B):
        qs, kt, vt = _prep_core(q[b], k[b], v[b])
        in_maps.append({"qs": qs, "kt": kt, "vt": vt, "cst": cst})

    trace = os.environ.get("KERNEL_TRACE", "0") == "1"
    if trace:
        _install_ntff_hook()
    res = run_bass_kernel_spmd(nc, in_maps, core_ids=list(range(B)), trace=trace)
    last_exec_time_ns = res.exec_time_ns

    outs = []
    for b in range(B):
        arr = np.asarray(res.results[b]["out"], dtype=np.float32)
        arr = arr.reshape(4, C, NBLK, W).transpose(1, 2, 0, 3).reshape(C, H, W)
        outs.append(arr)
    return np.stack(outs).astype(np.float32)
